# revision 20
# baseline (speedup 1.0000x reference)
"""LoRA layer kernel for Trainium2 (Bass/Tile), data-parallel over 8 NeuronCores.

Math:  out = (x @ B) @ A * (32/16)   with x [4,2048,4096], B [4096,16], A [16,4096].

Design (HBM-bound problem: ~8 MB in + ~8 MB out per core at f16):
  - Flatten tokens (4*2048=8192), shard 1024 tokens per core (data parallel).
  - x pre-tiled PARTITION-MAJOR on host as [ntb, 128, NB, tb] f16 so each
    half-block load is 128 fully-contiguous 8 KB descriptors (tiny strided
    descriptors were the original bottleneck: ~27% HBM efficiency).
  - Dispatch order on the sync queue: tiny B first (it gates every mm1
    LDWEIGHTS), then x block 0 in halves, then compact A ([16, 4096],
    replicated on-chip — not DMA'd 8x), then the remaining x blocks.
  - x tile pool is double-buffered ONLY (bufs=2): loads are issued
    just-in-time, which also stops the static scheduler from hoisting
    mm1 of later blocks ahead of mm2 of earlier ones (that hoisting
    stalls the PE on DMA and delays stores).
  - mm1: 4-way column-group packed fp16 matmuls; chunk 4k+g accumulates
    into PSUM partitions [32g, 32g+16).
  - The 4 col-group partials are folded with a DVE add chain (one PSUM
    operand per op) straight into mm2's row-group weight layout.
  - mm2: fp16, row-group packed over token subtiles (concurrent PE row
    strips). PSUM->SBUF output copies split even-subtile->DVE,
    odd-subtile->ACT so each store gates on exactly one engine.
  - Stores stream from the sync engine in o-halves so output drains
    while later chunks compute.
"""

import os
import numpy as np

IN = 4096
OUT = 4096
R = 16
N_CORES = 8
SCALE = 32.0 / 16.0
P = 128
NB = IN // P  # 32 contraction chunks


def _install_profile_hook():
    """Best-effort: register the axon NTFF profiling hook that this image's
    `antenv` package is missing, so run_bass_kernel_spmd(trace=True) can
    return exec_time_ns. Harmless no-op when anything is unavailable."""
    try:
        import sys
        import types

        if "antenv.axon_hooks" in sys.modules:
            return
        try:
            import antenv  # noqa: F401
        except ImportError:
            return
        mod = types.ModuleType("antenv.axon_hooks")
        mod._hook = None

        def set_axon_ntff_profile_hook(h):
            mod._hook = h

        def get_axon_ntff_profile_hook():
            return mod._hook

        mod.set_axon_ntff_profile_hook = set_axon_ntff_profile_hook
        mod.get_axon_ntff_profile_hook = get_axon_ntff_profile_hook
        sys.modules["antenv.axon_hooks"] = mod
        import antenv as _antenv

        _antenv.axon_hooks = mod

        so_path = "/opt/axon/libaxon_pjrt.so"
        if os.path.exists(so_path):
            try:
                from trn_agent_boot.trn_boot import _ntff_profile_via_ctypes

                hook = _ntff_profile_via_ctypes(so_path)
                if hook is not None:
                    mod._hook = hook
            except Exception:
                pass
    except Exception:
        pass


_install_profile_hook()

_NC_CACHE = {}


def build_nc(tok, tb=256):
    """Build + compile the per-core Bass program for `tok` tokens/core."""
    key = (tok, tb)
    if key in _NC_CACHE:
        return _NC_CACHE[key]

    import concourse.bacc as bacc
    import concourse.tile as tile
    from concourse import mybir

    f32 = mybir.dt.float32
    f16 = mybir.dt.float16
    tb = min(tb, tok)
    assert tok % tb == 0 and tb % P == 0
    ntb = tok // tb
    nst = tb // P  # token subtiles per block

    nc = bacc.Bacc("TRN2", target_bir_lowering=False, debug=False)
    xT = nc.dram_tensor("xT", [ntb, P, NB, tb], f16, kind="ExternalInput").ap()
    Bt = nc.dram_tensor("Bt", [P, NB, 2 * R], f16, kind="ExternalInput").ap()
    Af = nc.dram_tensor("Af", [R, OUT], f16, kind="ExternalInput").ap()
    out = nc.dram_tensor("out", [tok, OUT], f16, kind="ExternalOutput").ap()

    with tile.TileContext(nc) as tc:
        with (
            tc.tile_pool(name="const", bufs=1) as const_pool,
            tc.tile_pool(name="xin", bufs=min(2, ntb)) as x_pool,
            tc.tile_pool(name="xbt", bufs=2) as xbt_pool,
            tc.tile_pool(name="ps1", bufs=2, space="PSUM") as ps1,
            tc.tile_pool(name="ps2", bufs=6, space="PSUM") as ps2,
            tc.tile_pool(name="osb", bufs=2) as out_pool,
        ):
            xT_sbs = [
                x_pool.tile([P, NB, tb], f16, name=f"x{i}", tag="x")
                for i in range(ntb)
            ]
            # tiny consts first: B gates every mm1 LDWEIGHTS, S gates the
            # selector — never queue them behind megabytes of x
            B_sb = const_pool.tile([P, NB, 2 * R], f16)
            nc.sync.dma_start(out=B_sb[:], in_=Bt[:])
            # x block 0 next (in halves so mm1 starts on the first half)
            nq = NB // 2
            for q in range(2):
                nc.sync.dma_start(
                    out=xT_sbs[0][:, q * nq : (q + 1) * nq, :],
                    in_=xT[0, :, q * nq : (q + 1) * nq, :],
                )
            # A loaded compact, replicated on-chip to rows 32g+r (gates mm2
            # only, so it can sit behind x block 0)
            A_sb = const_pool.tile([P, OUT], f16)
            nc.sync.dma_start(out=A_sb[:R, :], in_=Af[:])
            for g in range(1, 4):
                nc.vector.tensor_copy(A_sb[32 * g : 32 * g + R, :], A_sb[:R, :])

            for tbi in range(1, ntb):
                for q in range(2):
                    nc.sync.dma_start(
                        out=xT_sbs[tbi][:, q * nq : (q + 1) * nq, :],
                        in_=xT[tbi, :, q * nq : (q + 1) * nq, :],
                    )

            # mm2 runs on GROUPS of blocks: up to 4 token subtiles (xbT rows
            # spread across 4 PE row strips) so the packed matmuls run 4-way
            # concurrent and the per-matmul fixed cost amortizes.
            gsz = 1  # blocks per mm2 group (grouping measured slower: stores back-load)
            for g0 in range(0, ntb, gsz):
                blocks = list(range(g0, min(g0 + gsz, ntb)))
                nst_tot = len(blocks) * nst
                xbt_sb = xbt_pool.tile([P, P], f16, tag="xbt")
                for bi, tbi in enumerate(blocks):
                    xT_sb = xT_sbs[tbi]
                    # mm1, 4-way column-group packed
                    ps_part = ps1.tile([P, tb], f32)
                    for c8 in range(NB // 4):
                        for g in range(4):
                            c = c8 * 4 + g
                            nc.tensor.matmul(
                                ps_part[32 * g : 32 * g + 2 * R, :],
                                lhsT=B_sb[:, c, :],
                                rhs=xT_sb[:, c, :],
                                start=(c8 == 0),
                                stop=(c8 == NB // 4 - 1),
                                tile_position=(0, 32 * g),
                                skip_group_check=True,
                            )
                    # fold the 4 col-group partials with a DVE chain (only
                    # one PSUM operand is legal per op), landing straight in
                    # mm2's row-group weight layout
                    f32 = ps_part.dtype
                    pa = xbt_pool.tile([R, tb], f32, name=f"pa{tbi}", tag="pa")
                    pb = xbt_pool.tile([R, tb], f32, name=f"pb{tbi}", tag="pb")
                    nc.vector.tensor_copy(pa[:], ps_part[0:R, :])
                    nc.vector.tensor_add(pa[:], ps_part[32 : 32 + R, :], pa[:])
                    nc.vector.tensor_add(pb[:], ps_part[64 : 64 + R, :], pa[:])
                    for j in range(nst):
                        st = bi * nst + j
                        nc.vector.tensor_add(
                            xbt_sb[32 * st : 32 * st + R, :],
                            ps_part[96 : 96 + R, j * P : (j + 1) * P],
                            pb[:, j * P : (j + 1) * P],
                        )

                o_sbs = [
                    out_pool.tile([P, OUT], f16, name=f"osb{st}_{g0}", tag=f"osb{st}")
                    for st in range(nst_tot)
                ]
                for o in range(OUT // 512):
                    for st in range(nst_tot):
                        ps_o = ps2.tile([P, 512], f32, tag="ps2")
                        nc.tensor.matmul(
                            ps_o[:],
                            lhsT=xbt_sb[32 * st : 32 * st + R, :],
                            rhs=A_sb[32 * st : 32 * st + R, o * 512 : (o + 1) * 512],
                            start=True,
                            stop=True,
                            tile_position=(32 * st, 0),
                            skip_group_check=True,
                        )
                        # even subtiles -> DVE, odd -> ACT: each store gates
                        # on exactly one copy engine
                        dst = o_sbs[st][:, o * 512 : (o + 1) * 512]
                        if st % 2 == 0:
                            nc.vector.tensor_copy(dst, ps_o[:])
                        else:
                            nc.scalar.activation(
                                dst, ps_o[:], mybir.ActivationFunctionType.Copy
                            )
                # stores dispatched from the (idle) sync engine, in o-halves
                # so the first half drains while the second computes
                for st in range(nst_tot):
                    t0 = g0 * tb + st * P
                    for h in range(2):
                        oh = h * (OUT // 2)
                        nc.sync.dma_start(
                            out=out[t0 : t0 + P, oh : oh + OUT // 2],
                            in_=o_sbs[st][:, oh : oh + OUT // 2],
                        )

    nc.compile()
    _NC_CACHE[key] = nc
    return nc


TB = 256


def make_in_maps(x, lora_A, lora_B, n_cores=N_CORES):
    x = np.asarray(x, dtype=np.float32)
    A = np.asarray(lora_A, dtype=np.float32)
    B = np.asarray(lora_B, dtype=np.float32)
    xf = x.reshape(-1, IN)
    ntok = xf.shape[0] // n_cores
    tb = min(TB, ntok)
    A_scaled = (A * np.float32(SCALE)).astype(np.float16)
    B_resh = np.zeros((P, NB, 2 * R), dtype=np.float16)
    B_resh[:, :, :R] = B.reshape(NB, P, R).transpose(1, 0, 2)
    in_maps = []
    for c in range(n_cores):
        shard = xf[c * ntok : (c + 1) * ntok]
        # pre-tile partition-major: [ntb, 128, NB, tb];
        # xt[tbi, p, c, t] = shard[tbi*tb + t, c*128 + p]
        xt = np.ascontiguousarray(
            shard.reshape(ntok // tb, tb, NB, P).transpose(0, 3, 2, 1),
            dtype=np.float16,
        )
        in_maps.append(
            {
                "xT": xt,
                "Bt": B_resh,
                "Af": A_scaled,
            }
        )
    return in_maps, ntok


def kernel_with_results(x, lora_A, lora_B, trace=False, **kwargs):
    from concourse.bass_utils import run_bass_kernel_spmd

    in_maps, ntok = make_in_maps(x, lora_A, lora_B)
    nc = build_nc(ntok, tb=TB)
    res = run_bass_kernel_spmd(nc, in_maps, list(range(N_CORES)), trace=trace, **kwargs)
    out = np.concatenate([r["out"] for r in res.results], axis=0).astype(np.float32)
    return out.reshape(np.asarray(x).shape[:-1] + (OUT,)), res


def kernel(x, lora_A, lora_B):
    out, _ = kernel_with_results(x, lora_A, lora_B)
    return out


# revision 21
# speedup vs baseline: 1.0214x; 1.0214x over previous
"""LoRA layer kernel for Trainium2 (Bass/Tile), data-parallel over 8 NeuronCores.

Math:  out = (x @ B) @ A * (32/16)   with x [4,2048,4096], B [4096,16], A [16,4096].

Design (HBM-bound problem: ~8 MB in + ~8 MB out per core at f16):
  - Flatten tokens (4*2048=8192), shard 1024 tokens per core (data parallel).
  - x pre-tiled PARTITION-MAJOR on host as [ntb, 128, NB, tb] f16 so each
    half-block load is 128 fully-contiguous 8 KB descriptors (tiny strided
    descriptors were the original bottleneck: ~27% HBM efficiency).
  - Dispatch order on the sync queue: tiny B first (it gates every mm1
    LDWEIGHTS), then x block 0 in halves, then compact A ([16, 4096],
    replicated on-chip — not DMA'd 8x), then the remaining x blocks.
  - x tile pool is double-buffered ONLY (bufs=2): loads are issued
    just-in-time, which also stops the static scheduler from hoisting
    mm1 of later blocks ahead of mm2 of earlier ones (that hoisting
    stalls the PE on DMA and delays stores).
  - mm1: 4-way column-group packed fp16 matmuls; chunk 4k+g accumulates
    into PSUM partitions [32g, 32g+16).
  - The 4 col-group partials are folded with a DVE add chain (one PSUM
    operand per op) straight into mm2's row-group weight layout.
  - mm2: fp16, row-group packed over token subtiles (concurrent PE row
    strips). PSUM->SBUF output copies split even-subtile->DVE,
    odd-subtile->ACT so each store gates on exactly one engine.
  - Stores stream from the sync engine in o-halves so output drains
    while later chunks compute.
"""

import os
import numpy as np

IN = 4096
OUT = 4096
R = 16
N_CORES = 8
SCALE = 32.0 / 16.0
P = 128
NB = IN // P  # 32 contraction chunks


def _install_profile_hook():
    """Best-effort: register the axon NTFF profiling hook that this image's
    `antenv` package is missing, so run_bass_kernel_spmd(trace=True) can
    return exec_time_ns. Harmless no-op when anything is unavailable."""
    try:
        import sys
        import types

        if "antenv.axon_hooks" in sys.modules:
            return
        try:
            import antenv  # noqa: F401
        except ImportError:
            return
        mod = types.ModuleType("antenv.axon_hooks")
        mod._hook = None

        def set_axon_ntff_profile_hook(h):
            mod._hook = h

        def get_axon_ntff_profile_hook():
            return mod._hook

        mod.set_axon_ntff_profile_hook = set_axon_ntff_profile_hook
        mod.get_axon_ntff_profile_hook = get_axon_ntff_profile_hook
        sys.modules["antenv.axon_hooks"] = mod
        import antenv as _antenv

        _antenv.axon_hooks = mod

        so_path = "/opt/axon/libaxon_pjrt.so"
        if os.path.exists(so_path):
            try:
                from trn_agent_boot.trn_boot import _ntff_profile_via_ctypes

                hook = _ntff_profile_via_ctypes(so_path)
                if hook is not None:
                    mod._hook = hook
            except Exception:
                pass
    except Exception:
        pass


_install_profile_hook()

_NC_CACHE = {}


def build_nc(tok, tb=256):
    """Build + compile the per-core Bass program for `tok` tokens/core."""
    key = (tok, tb)
    if key in _NC_CACHE:
        return _NC_CACHE[key]

    import concourse.bacc as bacc
    import concourse.tile as tile
    from concourse import mybir

    f32 = mybir.dt.float32
    f16 = mybir.dt.float16
    tb = min(tb, tok)
    assert tok % tb == 0 and tb % P == 0
    ntb = tok // tb
    nst = tb // P  # token subtiles per block

    nc = bacc.Bacc("TRN2", target_bir_lowering=False, debug=False)
    xT = nc.dram_tensor("xT", [ntb, P, NB, tb], f16, kind="ExternalInput").ap()
    Bt = nc.dram_tensor("Bt", [P, NB, 2 * R], f16, kind="ExternalInput").ap()
    Af = nc.dram_tensor("Af", [R, OUT], f16, kind="ExternalInput").ap()
    out = nc.dram_tensor("out", [tok, OUT], f16, kind="ExternalOutput").ap()

    with tile.TileContext(nc) as tc:
        with (
            tc.tile_pool(name="const", bufs=1) as const_pool,
            tc.tile_pool(name="xin", bufs=min(2, ntb)) as x_pool,
            tc.tile_pool(name="xbt", bufs=2) as xbt_pool,
            tc.tile_pool(name="ps1", bufs=2, space="PSUM") as ps1,
            tc.tile_pool(name="ps2", bufs=6, space="PSUM") as ps2,
            tc.tile_pool(name="osb", bufs=2) as out_pool,
        ):
            xT_sbs = [
                x_pool.tile([P, NB, tb], f16, name=f"x{i}", tag="x")
                for i in range(ntb)
            ]
            # tiny consts first: B gates every mm1 LDWEIGHTS — never queue
            # it behind megabytes of x
            B_sb = const_pool.tile([P, NB, 2 * R], f16)
            nc.sync.dma_start(out=B_sb[:], in_=Bt[:])
            # PE warm-up: ~4us of dummy matmuls while the PE would sit idle
            # waiting for x block 0 — releases the HAM clock throttle
            # (4/8 -> 8/8) so the real matmuls run at 2.4 GHz, not 1.2
            warm_sb = const_pool.tile([P, 512], f16)
            nc.vector.memset(warm_sb[:], 0.0)
            for _ in range(10):
                ps_w = ps2.tile([P, 512], f32, tag="ps2")
                nc.tensor.matmul(
                    ps_w[:],
                    lhsT=warm_sb[:, :P],
                    rhs=warm_sb[:],
                    start=True,
                    stop=True,
                    skip_group_check=True,
                )
            # x block 0 next (in halves so mm1 starts on the first half)
            nq = NB // 2
            for q in range(2):
                nc.sync.dma_start(
                    out=xT_sbs[0][:, q * nq : (q + 1) * nq, :],
                    in_=xT[0, :, q * nq : (q + 1) * nq, :],
                )
            # A loaded compact, replicated on-chip to rows 32g+r (gates mm2
            # only, so it can sit behind x block 0)
            A_sb = const_pool.tile([P, OUT], f16)
            nc.sync.dma_start(out=A_sb[:R, :], in_=Af[:])
            for g in range(1, 4):
                nc.vector.tensor_copy(A_sb[32 * g : 32 * g + R, :], A_sb[:R, :])

            for tbi in range(1, ntb):
                for q in range(2):
                    nc.sync.dma_start(
                        out=xT_sbs[tbi][:, q * nq : (q + 1) * nq, :],
                        in_=xT[tbi, :, q * nq : (q + 1) * nq, :],
                    )

            # mm2 runs on GROUPS of blocks: up to 4 token subtiles (xbT rows
            # spread across 4 PE row strips) so the packed matmuls run 4-way
            # concurrent and the per-matmul fixed cost amortizes.
            gsz = 1  # blocks per mm2 group (grouping measured slower: stores back-load)
            for g0 in range(0, ntb, gsz):
                blocks = list(range(g0, min(g0 + gsz, ntb)))
                nst_tot = len(blocks) * nst
                xbt_sb = xbt_pool.tile([P, P], f16, tag="xbt")
                for bi, tbi in enumerate(blocks):
                    xT_sb = xT_sbs[tbi]
                    # mm1, 4-way column-group packed
                    ps_part = ps1.tile([P, tb], f32)
                    for c8 in range(NB // 4):
                        for g in range(4):
                            c = c8 * 4 + g
                            nc.tensor.matmul(
                                ps_part[32 * g : 32 * g + 2 * R, :],
                                lhsT=B_sb[:, c, :],
                                rhs=xT_sb[:, c, :],
                                start=(c8 == 0),
                                stop=(c8 == NB // 4 - 1),
                                tile_position=(0, 32 * g),
                                skip_group_check=True,
                            )
                    # fold the 4 col-group partials with a DVE chain (only
                    # one PSUM operand is legal per op), landing straight in
                    # mm2's row-group weight layout
                    f32 = ps_part.dtype
                    pa = xbt_pool.tile([R, tb], f32, name=f"pa{tbi}", tag="pa")
                    pb = xbt_pool.tile([R, tb], f32, name=f"pb{tbi}", tag="pb")
                    nc.vector.tensor_copy(pa[:], ps_part[0:R, :])
                    nc.vector.tensor_add(pa[:], ps_part[32 : 32 + R, :], pa[:])
                    nc.vector.tensor_add(pb[:], ps_part[64 : 64 + R, :], pa[:])
                    for j in range(nst):
                        st = bi * nst + j
                        nc.vector.tensor_add(
                            xbt_sb[32 * st : 32 * st + R, :],
                            ps_part[96 : 96 + R, j * P : (j + 1) * P],
                            pb[:, j * P : (j + 1) * P],
                        )

                o_sbs = [
                    out_pool.tile([P, OUT], f16, name=f"osb{st}_{g0}", tag=f"osb{st}")
                    for st in range(nst_tot)
                ]
                for o in range(OUT // 512):
                    for st in range(nst_tot):
                        ps_o = ps2.tile([P, 512], f32, tag="ps2")
                        nc.tensor.matmul(
                            ps_o[:],
                            lhsT=xbt_sb[32 * st : 32 * st + R, :],
                            rhs=A_sb[32 * st : 32 * st + R, o * 512 : (o + 1) * 512],
                            start=True,
                            stop=True,
                            tile_position=(32 * st, 0),
                            skip_group_check=True,
                        )
                        # even subtiles -> DVE, odd -> ACT: each store gates
                        # on exactly one copy engine
                        dst = o_sbs[st][:, o * 512 : (o + 1) * 512]
                        if st % 2 == 0:
                            nc.vector.tensor_copy(dst, ps_o[:])
                        else:
                            nc.scalar.activation(
                                dst, ps_o[:], mybir.ActivationFunctionType.Copy
                            )
                # stores dispatched from the (idle) sync engine, in o-halves
                # so the first half drains while the second computes
                for st in range(nst_tot):
                    t0 = g0 * tb + st * P
                    for h in range(2):
                        oh = h * (OUT // 2)
                        nc.sync.dma_start(
                            out=out[t0 : t0 + P, oh : oh + OUT // 2],
                            in_=o_sbs[st][:, oh : oh + OUT // 2],
                        )

    nc.compile()
    _NC_CACHE[key] = nc
    return nc


TB = 256


def make_in_maps(x, lora_A, lora_B, n_cores=N_CORES):
    x = np.asarray(x, dtype=np.float32)
    A = np.asarray(lora_A, dtype=np.float32)
    B = np.asarray(lora_B, dtype=np.float32)
    xf = x.reshape(-1, IN)
    ntok = xf.shape[0] // n_cores
    tb = min(TB, ntok)
    A_scaled = (A * np.float32(SCALE)).astype(np.float16)
    B_resh = np.zeros((P, NB, 2 * R), dtype=np.float16)
    B_resh[:, :, :R] = B.reshape(NB, P, R).transpose(1, 0, 2)
    in_maps = []
    for c in range(n_cores):
        shard = xf[c * ntok : (c + 1) * ntok]
        # pre-tile partition-major: [ntb, 128, NB, tb];
        # xt[tbi, p, c, t] = shard[tbi*tb + t, c*128 + p]
        xt = np.ascontiguousarray(
            shard.reshape(ntok // tb, tb, NB, P).transpose(0, 3, 2, 1),
            dtype=np.float16,
        )
        in_maps.append(
            {
                "xT": xt,
                "Bt": B_resh,
                "Af": A_scaled,
            }
        )
    return in_maps, ntok


def kernel_with_results(x, lora_A, lora_B, trace=False, **kwargs):
    from concourse.bass_utils import run_bass_kernel_spmd

    in_maps, ntok = make_in_maps(x, lora_A, lora_B)
    nc = build_nc(ntok, tb=TB)
    res = run_bass_kernel_spmd(nc, in_maps, list(range(N_CORES)), trace=trace, **kwargs)
    out = np.concatenate([r["out"] for r in res.results], axis=0).astype(np.float32)
    return out.reshape(np.asarray(x).shape[:-1] + (OUT,)), res


def kernel(x, lora_A, lora_B):
    out, _ = kernel_with_results(x, lora_A, lora_B)
    return out
